# revision 15
# baseline (speedup 1.0000x reference)
"""Multi-head attention (B=4, S=2048, H=1024, 16 heads) on 8 trn2 NeuronCores.

Sharding: core c handles batch b = c//2, head-group g = c%2 (8 heads each).
No cross-core communication; weights column-split by head group.

Per-core dataflow (matmul operands bf16 = 1 cycle/row on the PE;
fp32 PSUM accumulation everywhere):
  xT [1024, 2048]  (x[b].T, hidden on partitions, bf16)
  qT = (WqT.T @ xT)*0.125 + bq/8   [512, 2048]  (2 heads per 128-part tile)
  kT = WkT.T @ xT + bk             [512, 2048]
  v  = xT.T @ WvT                  [2048, 512]  per s-chunk with a ones
                                   column appended per head -> [128, 8, 65]
  per head pair (A, B), per i-chunk (512 q positions), per j-chunk (128 k pos):
    S^T[j, i] = kT_h[:, j].T @ qT_h[:, i]   (A, B) in PE row groups (K=64)
    E = exp(S^T)                            one ACT op FD=1024, psum -> sbuf
    avX[0:65, :] += [v_h | 1][0:64].T  @ E[0:64]    row-split AV: all matmuls
    avY[0:65, :] += [v_h | 1][64:128].T @ E[64:128] are 64-row pairs, so the
                                            PE array stays fully packed and
                                            LDWEIGHTS hide under the opposite
                                            row group; row 64 = softmax sums
  tail: av = avX + avY; recip = 1/av[64] broadcast over 64 partitions (GpSimd)
        out^T = av[0:64] * recip + bv  -> DMA to outT [512, 2048] fp32
Projections for head-pairs 1..3 are emitted between attention i-chunks so the
ScalarE exp stream (the critical path) never starves.
Host transposes outT back into out[b][:, g*512:(g+1)*512].

Set _DTYPE_FALLBACK=True for a float32r variant (~2x slower, ~10x less error).
"""
import numpy as np
from contextlib import ExitStack

B = 4
SEQ = 2048
HIDDEN = 1024
HEADS = 16
HEAD = 64
NCORES = 8
HPC = HEADS // 2          # heads per core = 8
MT = 4                    # 128-row m-tiles of the 512 per-core features
KT = HIDDEN // 128        # 8 contraction tiles for projections
ST = SEQ // 128           # 16 seq tiles of 128
NI = 4                    # i-chunks of 512 q positions
NJ = 16                   # j-chunks of 128 k positions

_DTYPE_FALLBACK = False   # True -> float32r operands (slower, more precise)

_CACHE: dict = {}


def _build():
    import concourse.bacc as bacc
    import concourse.tile as tile
    from concourse import mybir

    MMDT = mybir.dt.float32r if _DTYPE_FALLBACK else mybir.dt.bfloat16
    F32 = mybir.dt.float32
    AF = mybir.ActivationFunctionType
    ALU = mybir.AluOpType

    nc = bacc.Bacc("TRN2", target_bir_lowering=False, debug=False,
                   num_devices=NCORES)
    xT = nc.dram_tensor("xT", [HIDDEN, SEQ], MMDT, kind="ExternalInput").ap()
    wqT = nc.dram_tensor("wqT", [HIDDEN, 512], MMDT, kind="ExternalInput").ap()
    wkT = nc.dram_tensor("wkT", [HIDDEN, 512], MMDT, kind="ExternalInput").ap()
    wvT = nc.dram_tensor("wvT", [HIDDEN, 512], MMDT, kind="ExternalInput").ap()
    bq8 = nc.dram_tensor("bq8", [128, MT], F32, kind="ExternalInput").ap()
    bk4 = nc.dram_tensor("bk4", [128, MT], F32, kind="ExternalInput").ap()
    bv4 = nc.dram_tensor("bv4", [128, MT], F32, kind="ExternalInput").ap()
    outT = nc.dram_tensor("outT", [512, SEQ], F32, kind="ExternalOutput").ap()

    with tile.TileContext(nc) as tc, ExitStack() as ctx:
        consts = ctx.enter_context(tc.tile_pool(name="consts", bufs=1))
        bq8_sb = consts.tile([128, MT], F32)
        nc.sync.dma_start(bq8_sb[:], bq8[:])
        bk4_sb = consts.tile([128, MT], F32)
        nc.sync.dma_start(bk4_sb[:], bk4[:])
        bv4_sb = consts.tile([128, MT], F32)
        nc.sync.dma_start(bv4_sb[:], bv4[:])

        qk = ctx.enter_context(tc.tile_pool(name="qk", bufs=1))
        qT_sb = qk.tile([128, MT, SEQ], MMDT, tag="q")
        kT_sb = qk.tile([128, MT, SEQ], MMDT, tag="k")
        v_sb = qk.tile([128, ST, HPC, HEAD + 1], MMDT, tag="v")
        ones_sb = consts.tile([128, ST * HPC], F32)
        nc.gpsimd.memset(ones_sb[:], 1.0)
        nc.vector.tensor_copy(
            v_sb[:, :, :, HEAD:HEAD + 1],
            ones_sb[:].rearrange("p (a b c) -> p a b c", a=ST, b=HPC, c=1))

        # input tiles: one tile per k-chunk so matmuls only depend on the
        # DMA that actually feeds them (first matmul starts ~2us in)
        xt_pool = ctx.enter_context(tc.tile_pool(name="xt", bufs=1))
        w_pool = ctx.enter_context(tc.tile_pool(name="w", bufs=1))
        import os as _os
        _split = not _os.environ.get("K_MONO_TILES")
        xt = []
        if _split:
            for k in range(KT):
                t = xt_pool.tile([128, SEQ], MMDT, name=f"xt{k}", tag=f"xt{k}")
                for c in range(4):
                    nc.sync.dma_start(
                        t[:, c * 512:(c + 1) * 512],
                        xT[k * 128:(k + 1) * 128, c * 512:(c + 1) * 512])
                xt.append(t)
        else:
            xt_all = xt_pool.tile([128, KT, SEQ], MMDT, name="xt_all")
            for k in range(KT):
                nc.sync.dma_start(xt_all[:, k, :], xT[k * 128:(k + 1) * 128, :])
                xt.append(xt_all[:, k, :])

        wts = {}

        def load_w(which, dram):
            tiles = []
            if _split:
                for k in range(KT):
                    t = w_pool.tile([128, 512], MMDT, name=f"w{which}{k}",
                                    tag=f"w{which}{k}")
                    for c in range(2):
                        nc.sync.dma_start(
                            t[:, c * 256:(c + 1) * 256],
                            dram[k * 128:(k + 1) * 128,
                                 c * 256:(c + 1) * 256])
                    tiles.append(t)
            else:
                w_all = w_pool.tile([128, KT, 512], MMDT, name=f"w{which}_all",
                                    tag=f"w{which}_all")
                for k in range(KT):
                    nc.sync.dma_start(w_all[:, k, :],
                                      dram[k * 128:(k + 1) * 128, :])
                    tiles.append(w_all[:, k, :])
            wts[which] = tiles

        load_w("q", wqT)
        load_w("k", wkT)
        load_w("v", wvT)

        def proj_copy(which, m, n, ps):
            dst = qT_sb if which == "q" else kT_sb
            if which == "q":
                nc.vector.tensor_scalar(
                    dst[:, m, n * 512:(n + 1) * 512], ps[:],
                    0.125, bq8_sb[:, m:m + 1], ALU.mult, ALU.add)
            else:
                nc.vector.tensor_scalar_add(
                    dst[:, m, n * 512:(n + 1) * 512], ps[:],
                    bk4_sb[:, m:m + 1])

        # lead-in: q/k projections for pair 0, then all of v (pp8 scope,
        # released before the attention psum pools allocate)
        with tc.tile_pool(name="pp8", bufs=8, space="PSUM") as pp8:
            for which in ("q", "k"):
                w_t = wts[which]
                pss = []
                for n in range(NI):
                    pss.append(pp8.tile([128, 512], F32, tag="pp8",
                                        name=f"pp{which}0n{n}"))
                for k in range(KT):
                    for n in range(NI):
                        nc.tensor.matmul(
                            pss[n][:], w_t[k][:, 0:128],
                            xt[k][:, n * 512:(n + 1) * 512],
                            start=(k == 0), stop=(k == KT - 1))
                for n in range(NI):
                    proj_copy(which, 0, n, pss[n])
            for wave in (list(range(8)), list(range(8, ST))):
                pss = {}
                for s in wave:
                    pss[s] = pp8.tile([128, 512], F32, tag="pp8",
                                      name=f"ppv{s}")
                for k in range(KT):
                    for s in wave:
                        nc.tensor.matmul(
                            pss[s][:], xt[k][:, s * 128:(s + 1) * 128],
                            wts["v"][k][:],
                            start=(k == 0), stop=(k == KT - 1))
                for s in wave:
                    nc.vector.tensor_copy(
                        v_sb[:, s, :, 0:HEAD],
                        pss[s][:].rearrange("p (h d) -> p h d", h=HPC))

        # remaining projections, to be interleaved into the attention stream:
        # (which, m) blocks in the order they are needed
        import os
        pending = []
        for m in range(1, MT):
            pending.append(("q", m))
            pending.append(("k", m))
        _no_interleave = bool(os.environ.get("K_NO_INTERLEAVE"))

        es_pool = ctx.enter_context(tc.tile_pool(name="es", bufs=4))
        tail_pool = ctx.enter_context(tc.tile_pool(name="tail", bufs=2))
        sab_ps = ctx.enter_context(
            tc.tile_pool(name="sab", bufs=2, space="PSUM"))
        av_ps = ctx.enter_context(
            tc.tile_pool(name="avp", bufs=1, space="PSUM"))
        pp2 = ctx.enter_context(tc.tile_pool(name="pp2", bufs=2, space="PSUM"))


        def proj_block(which, m):
            # one m-tile of q/k projection as 2 k-outer waves of 2 psum
            # groups (weights shared between the 2 matmuls of each k step)
            w_t = wts[which]
            for nw in (0, 2):
                pss = []
                for n in (nw, nw + 1):
                    pss.append(pp2.tile([128, 512], F32, tag="pp2",
                                        name=f"p2{which}{m}n{n}"))
                for k in range(KT):
                    for d, n in enumerate((nw, nw + 1)):
                        nc.tensor.matmul(
                            pss[d][:], w_t[k][:, m * 128:(m + 1) * 128],
                            xt[k][:, n * 512:(n + 1) * 512],
                            start=(k == 0), stop=(k == KT - 1))
                for d, n in enumerate((nw, nw + 1)):
                    proj_copy(which, m, n, pss[d])

        if _no_interleave:
            while pending:
                _w, _m = pending.pop(0)
                proj_block(_w, _m)

        for p in range(MT):          # head pair (2p, 2p+1)
            for i in range(NI):      # q chunk of 512
                av = av_ps.tile([128, 1024], F32, tag="av")
                es_q = []
                for j in range(NJ + 1):  # k chunk of 128; AV lags S by one j
                    if j < NJ:
                        sab = sab_ps.tile([128, 1024], F32, tag="sab")
                        nc.tensor.matmul(
                            sab[:, 0:512],
                            kT_sb[0:64, p, j * 128:(j + 1) * 128],
                            qT_sb[0:64, p, i * 512:(i + 1) * 512],
                            start=True, stop=True)
                        nc.tensor.matmul(
                            sab[:, 512:1024],
                            kT_sb[64:128, p, j * 128:(j + 1) * 128],
                            qT_sb[64:128, p, i * 512:(i + 1) * 512],
                            start=True, stop=True)
                        es = es_pool.tile([128, 1024], MMDT, tag="es")
                        nc.scalar.activation(es[:], sab[:], AF.Exp)
                        es_q.append(es)
                    if j >= 1:
                        jj = j - 1
                        es = es_q[jj]
                        st, sp = (jj == 0), (jj == NJ - 1)
                        nc.tensor.matmul(
                            av[0:65, 0:512], v_sb[:, jj, 2 * p, :],
                            es[:, 0:512], start=st, stop=sp)
                        nc.tensor.matmul(
                            av[0:65, 512:1024], v_sb[:, jj, 2 * p + 1, :],
                            es[:, 512:1024], start=st, stop=sp)
                # tail: evacuate av to SBUF immediately so the psum bank
                # frees for the next i-chunk; everything else is SBUF-side
                av_sb = tail_pool.tile([65, 1024], F32, tag="avsb")
                nc.vector.tensor_copy(av_sb[:], av[0:65, :])
                # custom DVE / gpsimd ops require partition-0 inputs on HW:
                # stage the sums row into its own tile first
                sums = tail_pool.tile([1, 1024], F32, tag="sums")
                nc.vector.tensor_copy(sums[:], av_sb[64:65, :])
                recip = tail_pool.tile([1, 1024], F32, tag="recip")
                nc.vector.reciprocal_approx_fast(recip[:], sums[:])
                bcast = tail_pool.tile([64, 1024], F32, tag="bcast")
                nc.gpsimd.partition_broadcast(bcast[:], recip[:], channels=64)
                outm = tail_pool.tile([64, 1024], F32, tag="outm")
                nc.vector.tensor_tensor(outm[:], av_sb[0:64, :], bcast[:],
                                        ALU.mult)
                outf = tail_pool.tile([64, 1024], F32, tag="outf")
                nc.vector.tensor_scalar_add(outf[:, 0:512], outm[:, 0:512],
                                            bv4_sb[0:64, p:p + 1])
                nc.vector.tensor_scalar_add(outf[:, 512:1024],
                                            outm[:, 512:1024],
                                            bv4_sb[64:128, p:p + 1])
                nc.sync.dma_start(
                    outT[p * 128:p * 128 + 64, i * 512:(i + 1) * 512],
                    outf[:, 0:512])
                nc.sync.dma_start(
                    outT[p * 128 + 64:(p + 1) * 128, i * 512:(i + 1) * 512],
                    outf[:, 512:1024])
                # inject one pending projection block between i-chunks
                if pending and i % 2 == 1:
                    which, m = pending.pop(0)
                    proj_block(which, m)

    nc.compile()
    return nc


def kernel(x, Wq, bq, Wk, bk, Wv, bv):
    import ml_dtypes
    from concourse.bass_utils import run_bass_kernel_spmd

    mmdt = np.float32 if _DTYPE_FALLBACK else ml_dtypes.bfloat16

    x = np.asarray(x, dtype=np.float32)
    Wq = np.asarray(Wq, dtype=np.float32)
    bq = np.asarray(bq, dtype=np.float32)
    Wk = np.asarray(Wk, dtype=np.float32)
    bk = np.asarray(bk, dtype=np.float32)
    Wv = np.asarray(Wv, dtype=np.float32)
    bv = np.asarray(bv, dtype=np.float32)

    if "nc" not in _CACHE:
        _CACHE["nc"] = _build()
    nc = _CACHE["nc"]

    xTs = [np.ascontiguousarray(x[b].T).astype(mmdt) for b in range(B)]
    wT = {}
    for g in range(2):
        sl = slice(g * 512, (g + 1) * 512)
        wT[g] = (
            np.ascontiguousarray(Wq[sl, :].T).astype(mmdt),
            np.ascontiguousarray(Wk[sl, :].T).astype(mmdt),
            np.ascontiguousarray(Wv[sl, :].T).astype(mmdt),
            np.ascontiguousarray((bq[sl] * 0.125).reshape(MT, 128).T),
            np.ascontiguousarray(bk[sl].reshape(MT, 128).T),
            np.ascontiguousarray(bv[sl].reshape(MT, 128).T),
        )

    in_maps = []
    for c in range(NCORES):
        b, g = c // 2, c % 2
        wq_t, wk_t, wv_t, bq8, bk4, bv4 = wT[g]
        in_maps.append({
            "xT": xTs[b], "wqT": wq_t, "wkT": wk_t, "wvT": wv_t,
            "bq8": bq8, "bk4": bk4, "bv4": bv4,
        })

    res = run_bass_kernel_spmd(nc, in_maps, list(range(NCORES)),
                               **_CACHE.get("run_kwargs", {}))
    _CACHE["last_result"] = res

    out = np.empty((B, SEQ, HIDDEN), dtype=np.float32)
    for c in range(NCORES):
        b, g = c // 2, c % 2
        out[b, :, g * 512:(g + 1) * 512] = res.results[c]["outT"].T
    return out


# revision 16
# speedup vs baseline: 1.0047x; 1.0047x over previous
"""Multi-head attention (B=4, S=2048, H=1024, 16 heads) on 8 trn2 NeuronCores.

Sharding: core c handles batch b = c//2, head-group g = c%2 (8 heads each).
No cross-core communication; weights column-split by head group.

Per-core dataflow (matmul operands bf16 = 1 cycle/row on the PE;
fp32 PSUM accumulation everywhere):
  xT [1024, 2048]  (x[b].T, hidden on partitions, bf16)
  qT = (WqT.T @ xT)*0.125 + bq/8   [512, 2048]  (2 heads per 128-part tile)
  kT = WkT.T @ xT + bk             [512, 2048]
  v  = xT.T @ WvT                  [2048, 512]  per s-chunk with a ones
                                   column appended per head -> [128, 8, 65]
  per head pair (A, B), per i-chunk (512 q positions), per j-chunk (128 k pos):
    S^T[j, i] = kT_h[:, j].T @ qT_h[:, i]   (A, B) in PE row groups (K=64)
    E = exp(S^T)                            one ACT op FD=1024, psum -> sbuf
    avX[0:65, :] += [v_h | 1][0:64].T  @ E[0:64]    row-split AV: all matmuls
    avY[0:65, :] += [v_h | 1][64:128].T @ E[64:128] are 64-row pairs, so the
                                            PE array stays fully packed and
                                            LDWEIGHTS hide under the opposite
                                            row group; row 64 = softmax sums
  tail: av = avX + avY; recip = 1/av[64] broadcast over 64 partitions (GpSimd)
        out^T = av[0:64] * recip + bv  -> DMA to outT [512, 2048] fp32
Projections for head-pairs 1..3 are emitted between attention i-chunks so the
ScalarE exp stream (the critical path) never starves.
Host transposes outT back into out[b][:, g*512:(g+1)*512].

Set _DTYPE_FALLBACK=True for a float32r variant (~2x slower, ~10x less error).
"""
import numpy as np
from contextlib import ExitStack

B = 4
SEQ = 2048
HIDDEN = 1024
HEADS = 16
HEAD = 64
NCORES = 8
HPC = HEADS // 2          # heads per core = 8
MT = 4                    # 128-row m-tiles of the 512 per-core features
KT = HIDDEN // 128        # 8 contraction tiles for projections
ST = SEQ // 128           # 16 seq tiles of 128
NI = 4                    # i-chunks of 512 q positions
NJ = 16                   # j-chunks of 128 k positions

_DTYPE_FALLBACK = False   # True -> float32r operands (slower, more precise)

_CACHE: dict = {}


def _build():
    import concourse.bacc as bacc
    import concourse.tile as tile
    from concourse import mybir

    MMDT = mybir.dt.float32r if _DTYPE_FALLBACK else mybir.dt.bfloat16
    F32 = mybir.dt.float32
    AF = mybir.ActivationFunctionType
    ALU = mybir.AluOpType

    nc = bacc.Bacc("TRN2", target_bir_lowering=False, debug=False,
                   num_devices=NCORES)
    xT = nc.dram_tensor("xT", [HIDDEN, SEQ], MMDT, kind="ExternalInput").ap()
    wqT = nc.dram_tensor("wqT", [HIDDEN, 512], MMDT, kind="ExternalInput").ap()
    wkT = nc.dram_tensor("wkT", [HIDDEN, 512], MMDT, kind="ExternalInput").ap()
    wvT = nc.dram_tensor("wvT", [HIDDEN, 512], MMDT, kind="ExternalInput").ap()
    bq8 = nc.dram_tensor("bq8", [128, MT], F32, kind="ExternalInput").ap()
    bk4 = nc.dram_tensor("bk4", [128, MT], F32, kind="ExternalInput").ap()
    bv4 = nc.dram_tensor("bv4", [128, MT], F32, kind="ExternalInput").ap()
    outT = nc.dram_tensor("outT", [512, SEQ], F32, kind="ExternalOutput").ap()

    with tile.TileContext(nc) as tc, ExitStack() as ctx:
        consts = ctx.enter_context(tc.tile_pool(name="consts", bufs=1))
        bq8_sb = consts.tile([128, MT], F32)
        nc.sync.dma_start(bq8_sb[:], bq8[:])
        bk4_sb = consts.tile([128, MT], F32)
        nc.sync.dma_start(bk4_sb[:], bk4[:])
        bv4_sb = consts.tile([128, MT], F32)
        nc.sync.dma_start(bv4_sb[:], bv4[:])

        qk = ctx.enter_context(tc.tile_pool(name="qk", bufs=1))
        qT_sb = qk.tile([128, MT, SEQ], MMDT, tag="q")
        kT_sb = qk.tile([128, MT, SEQ], MMDT, tag="k")
        v_sb = qk.tile([128, ST, HPC, HEAD + 1], MMDT, tag="v")
        ones_sb = consts.tile([128, ST * HPC], F32)
        nc.gpsimd.memset(ones_sb[:], 1.0)
        nc.vector.tensor_copy(
            v_sb[:, :, :, HEAD:HEAD + 1],
            ones_sb[:].rearrange("p (a b c) -> p a b c", a=ST, b=HPC, c=1))

        # input tiles: one tile per k-chunk so matmuls only depend on the
        # DMA that actually feeds them (first matmul starts ~2us in)
        xt_pool = ctx.enter_context(tc.tile_pool(name="xt", bufs=1))
        w_pool = ctx.enter_context(tc.tile_pool(name="w", bufs=1))
        import os as _os
        _split = not _os.environ.get("K_MONO_TILES")
        xt = []
        if _split:
            for k in range(KT):
                t = xt_pool.tile([128, SEQ], MMDT, name=f"xt{k}", tag=f"xt{k}")
                for c in range(4):
                    nc.sync.dma_start(
                        t[:, c * 512:(c + 1) * 512],
                        xT[k * 128:(k + 1) * 128, c * 512:(c + 1) * 512])
                xt.append(t)
        else:
            xt_all = xt_pool.tile([128, KT, SEQ], MMDT, name="xt_all")
            for k in range(KT):
                nc.sync.dma_start(xt_all[:, k, :], xT[k * 128:(k + 1) * 128, :])
                xt.append(xt_all[:, k, :])

        wts = {}

        def load_w(which, dram):
            tiles = []
            if _split:
                for k in range(KT):
                    t = w_pool.tile([128, 512], MMDT, name=f"w{which}{k}",
                                    tag=f"w{which}{k}")
                    for c in range(2):
                        nc.sync.dma_start(
                            t[:, c * 256:(c + 1) * 256],
                            dram[k * 128:(k + 1) * 128,
                                 c * 256:(c + 1) * 256])
                    tiles.append(t)
            else:
                w_all = w_pool.tile([128, KT, 512], MMDT, name=f"w{which}_all",
                                    tag=f"w{which}_all")
                for k in range(KT):
                    nc.sync.dma_start(w_all[:, k, :],
                                      dram[k * 128:(k + 1) * 128, :])
                    tiles.append(w_all[:, k, :])
            wts[which] = tiles

        load_w("q", wqT)
        load_w("k", wkT)
        load_w("v", wvT)

        def proj_copy(which, m, n, ps):
            dst = qT_sb if which == "q" else kT_sb
            if which == "q":
                nc.vector.tensor_scalar(
                    dst[:, m, n * 512:(n + 1) * 512], ps[:],
                    0.125, bq8_sb[:, m:m + 1], ALU.mult, ALU.add)
            else:
                nc.vector.tensor_scalar_add(
                    dst[:, m, n * 512:(n + 1) * 512], ps[:],
                    bk4_sb[:, m:m + 1])

        # lead-in: q/k projections for pair 0, then all of v (pp8 scope,
        # released before the attention psum pools allocate)
        with tc.tile_pool(name="pp8", bufs=8, space="PSUM") as pp8:
            for which in ("q", "k"):
                w_t = wts[which]
                pss = []
                for n in range(NI):
                    pss.append(pp8.tile([128, 512], F32, tag="pp8",
                                        name=f"pp{which}0n{n}"))
                for k in range(KT):
                    for n in range(NI):
                        nc.tensor.matmul(
                            pss[n][:], w_t[k][:, 0:128],
                            xt[k][:, n * 512:(n + 1) * 512],
                            start=(k == 0), stop=(k == KT - 1))
                for n in range(NI):
                    proj_copy(which, 0, n, pss[n])
            for wave in (list(range(8)), list(range(8, ST))):
                pss = {}
                for s in wave:
                    pss[s] = pp8.tile([128, 512], F32, tag="pp8",
                                      name=f"ppv{s}")
                for k in range(KT):
                    for s in wave:
                        nc.tensor.matmul(
                            pss[s][:], xt[k][:, s * 128:(s + 1) * 128],
                            wts["v"][k][:],
                            start=(k == 0), stop=(k == KT - 1))
                for s in wave:
                    nc.vector.tensor_copy(
                        v_sb[:, s, :, 0:HEAD],
                        pss[s][:].rearrange("p (h d) -> p h d", h=HPC))

        # remaining projections, to be interleaved into the attention stream:
        # (which, m) blocks in the order they are needed
        import os
        pending = []
        for m in range(1, MT):
            pending.append(("q", m))
            pending.append(("k", m))
        _no_interleave = bool(os.environ.get("K_NO_INTERLEAVE"))

        es_pool = ctx.enter_context(tc.tile_pool(name="es", bufs=4))
        tail_pool = ctx.enter_context(tc.tile_pool(name="tail", bufs=2))
        sab_ps = ctx.enter_context(
            tc.tile_pool(name="sab", bufs=2, space="PSUM"))
        av_ps = ctx.enter_context(
            tc.tile_pool(name="avp", bufs=1, space="PSUM"))
        pp2 = ctx.enter_context(tc.tile_pool(name="pp2", bufs=2, space="PSUM"))


        def proj_block(which, m):
            # one m-tile of q/k projection as 2 k-outer waves of 2 psum
            # groups (weights shared between the 2 matmuls of each k step)
            w_t = wts[which]
            for nw in (0, 2):
                pss = []
                for n in (nw, nw + 1):
                    pss.append(pp2.tile([128, 512], F32, tag="pp2",
                                        name=f"p2{which}{m}n{n}"))
                for k in range(KT):
                    for d, n in enumerate((nw, nw + 1)):
                        nc.tensor.matmul(
                            pss[d][:], w_t[k][:, m * 128:(m + 1) * 128],
                            xt[k][:, n * 512:(n + 1) * 512],
                            start=(k == 0), stop=(k == KT - 1))
                for d, n in enumerate((nw, nw + 1)):
                    proj_copy(which, m, n, pss[d])

        if _no_interleave:
            while pending:
                _w, _m = pending.pop(0)
                proj_block(_w, _m)

        for p in range(MT):          # head pair (2p, 2p+1)
            for i in range(NI):      # q chunk of 512
                av = av_ps.tile([128, 1024], F32, tag="av")
                for j in range(NJ):  # k chunk of 128
                    sab = sab_ps.tile([128, 1024], F32, tag="sab")
                    nc.tensor.matmul(
                        sab[:, 0:512],
                        kT_sb[0:64, p, j * 128:(j + 1) * 128],
                        qT_sb[0:64, p, i * 512:(i + 1) * 512],
                        start=True, stop=True)
                    nc.tensor.matmul(
                        sab[:, 512:1024],
                        kT_sb[64:128, p, j * 128:(j + 1) * 128],
                        qT_sb[64:128, p, i * 512:(i + 1) * 512],
                        start=True, stop=True)
                    es = es_pool.tile([128, 1024], MMDT, tag="es")
                    nc.scalar.activation(es[:], sab[:], AF.Exp)
                    st, sp = (j == 0), (j == NJ - 1)
                    nc.tensor.matmul(
                        av[0:65, 0:512], v_sb[:, j, 2 * p, :],
                        es[:, 0:512], start=st, stop=sp)
                    nc.tensor.matmul(
                        av[0:65, 512:1024], v_sb[:, j, 2 * p + 1, :],
                        es[:, 512:1024], start=st, stop=sp)
                # tail: evacuate av to SBUF immediately so the psum bank
                # frees for the next i-chunk; everything else is SBUF-side
                av_sb = tail_pool.tile([65, 1024], F32, tag="avsb")
                nc.vector.tensor_copy(av_sb[:], av[0:65, :])
                # custom DVE / gpsimd ops require partition-0 inputs on HW:
                # stage the sums row into its own tile first
                sums = tail_pool.tile([1, 1024], F32, tag="sums")
                nc.vector.tensor_copy(sums[:], av_sb[64:65, :])
                recip = tail_pool.tile([1, 1024], F32, tag="recip")
                nc.vector.reciprocal_approx_fast(recip[:], sums[:])
                bcast = tail_pool.tile([64, 1024], F32, tag="bcast")
                nc.gpsimd.partition_broadcast(bcast[:], recip[:], channels=64)
                outm = tail_pool.tile([64, 1024], F32, tag="outm")
                nc.vector.tensor_tensor(outm[:], av_sb[0:64, :], bcast[:],
                                        ALU.mult)
                outf = tail_pool.tile([64, 1024], F32, tag="outf")
                nc.vector.tensor_scalar_add(outf[:, 0:512], outm[:, 0:512],
                                            bv4_sb[0:64, p:p + 1])
                nc.vector.tensor_scalar_add(outf[:, 512:1024],
                                            outm[:, 512:1024],
                                            bv4_sb[64:128, p:p + 1])
                nc.sync.dma_start(
                    outT[p * 128:p * 128 + 64, i * 512:(i + 1) * 512],
                    outf[:, 0:512])
                nc.sync.dma_start(
                    outT[p * 128 + 64:(p + 1) * 128, i * 512:(i + 1) * 512],
                    outf[:, 512:1024])
                # inject one pending projection block between i-chunks
                if pending and i % 2 == 1:
                    which, m = pending.pop(0)
                    proj_block(which, m)

    nc.compile()
    return nc


def kernel(x, Wq, bq, Wk, bk, Wv, bv):
    import ml_dtypes
    from concourse.bass_utils import run_bass_kernel_spmd

    mmdt = np.float32 if _DTYPE_FALLBACK else ml_dtypes.bfloat16

    x = np.asarray(x, dtype=np.float32)
    Wq = np.asarray(Wq, dtype=np.float32)
    bq = np.asarray(bq, dtype=np.float32)
    Wk = np.asarray(Wk, dtype=np.float32)
    bk = np.asarray(bk, dtype=np.float32)
    Wv = np.asarray(Wv, dtype=np.float32)
    bv = np.asarray(bv, dtype=np.float32)

    if "nc" not in _CACHE:
        _CACHE["nc"] = _build()
    nc = _CACHE["nc"]

    xTs = [np.ascontiguousarray(x[b].T).astype(mmdt) for b in range(B)]
    wT = {}
    for g in range(2):
        sl = slice(g * 512, (g + 1) * 512)
        wT[g] = (
            np.ascontiguousarray(Wq[sl, :].T).astype(mmdt),
            np.ascontiguousarray(Wk[sl, :].T).astype(mmdt),
            np.ascontiguousarray(Wv[sl, :].T).astype(mmdt),
            np.ascontiguousarray((bq[sl] * 0.125).reshape(MT, 128).T),
            np.ascontiguousarray(bk[sl].reshape(MT, 128).T),
            np.ascontiguousarray(bv[sl].reshape(MT, 128).T),
        )

    in_maps = []
    for c in range(NCORES):
        b, g = c // 2, c % 2
        wq_t, wk_t, wv_t, bq8, bk4, bv4 = wT[g]
        in_maps.append({
            "xT": xTs[b], "wqT": wq_t, "wkT": wk_t, "wvT": wv_t,
            "bq8": bq8, "bk4": bk4, "bv4": bv4,
        })

    res = run_bass_kernel_spmd(nc, in_maps, list(range(NCORES)),
                               **_CACHE.get("run_kwargs", {}))
    _CACHE["last_result"] = res

    out = np.empty((B, SEQ, HIDDEN), dtype=np.float32)
    for c in range(NCORES):
        b, g = c // 2, c % 2
        out[b, :, g * 512:(g + 1) * 512] = res.results[c]["outT"].T
    return out


# revision 17
# speedup vs baseline: 1.0463x; 1.0414x over previous
"""Multi-head attention (B=4, S=2048, H=1024, 16 heads) on 8 trn2 NeuronCores.

Sharding: core c handles batch b = c//2, head-group g = c%2 (8 heads each).
No cross-core communication; weights column-split by head group.

Per-core dataflow (matmul operands bf16 = 1 cycle/row on the PE;
fp32 PSUM accumulation everywhere):
  xT [1024, 2048]  (x[b].T, hidden on partitions, bf16)
  qT = (WqT.T @ xT)*0.125 + bq/8   [512, 2048]  (2 heads per 128-part tile)
  kT = WkT.T @ xT + bk             [512, 2048]
  v  = xT.T @ WvT                  [2048, 512]  per s-chunk with a ones
                                   column appended per head -> [128, 8, 65]
  per head pair (A, B), per i-chunk (512 q positions), per j-chunk (128 k pos):
    S^T[j, i] = kT_h[:, j].T @ qT_h[:, i]   (A, B) in PE row groups (K=64)
    E = exp(S^T)                            one ACT op FD=1024, psum -> sbuf
    avX[0:65, :] += [v_h | 1][0:64].T  @ E[0:64]    row-split AV: all matmuls
    avY[0:65, :] += [v_h | 1][64:128].T @ E[64:128] are 64-row pairs, so the
                                            PE array stays fully packed and
                                            LDWEIGHTS hide under the opposite
                                            row group; row 64 = softmax sums
  tail: av = avX + avY; recip = 1/av[64] broadcast over 64 partitions (GpSimd)
        out^T = av[0:64] * recip + bv  -> DMA to outT [512, 2048] fp32
Projections for head-pairs 1..3 are emitted between attention i-chunks so the
ScalarE exp stream (the critical path) never starves.
Host transposes outT back into out[b][:, g*512:(g+1)*512].

Set _DTYPE_FALLBACK=True for a float32r variant (~2x slower, ~10x less error).
"""
import numpy as np
from contextlib import ExitStack

B = 4
SEQ = 2048
HIDDEN = 1024
HEADS = 16
HEAD = 64
NCORES = 8
HPC = HEADS // 2          # heads per core = 8
MT = 4                    # 128-row m-tiles of the 512 per-core features
KT = HIDDEN // 128        # 8 contraction tiles for projections
ST = SEQ // 128           # 16 seq tiles of 128
NI = 4                    # i-chunks of 512 q positions
NJ = 16                   # j-chunks of 128 k positions

_DTYPE_FALLBACK = False   # True -> float32r operands (slower, more precise)

_CACHE: dict = {}


def _build():
    import concourse.bacc as bacc
    import concourse.tile as tile
    from concourse import mybir

    MMDT = mybir.dt.float32r if _DTYPE_FALLBACK else mybir.dt.bfloat16
    F32 = mybir.dt.float32
    AF = mybir.ActivationFunctionType
    ALU = mybir.AluOpType

    nc = bacc.Bacc("TRN2", target_bir_lowering=False, debug=False,
                   num_devices=NCORES)
    xT = nc.dram_tensor("xT", [HIDDEN, SEQ], MMDT, kind="ExternalInput").ap()
    wqT = nc.dram_tensor("wqT", [HIDDEN, 512], MMDT, kind="ExternalInput").ap()
    wkT = nc.dram_tensor("wkT", [HIDDEN, 512], MMDT, kind="ExternalInput").ap()
    wvT = nc.dram_tensor("wvT", [HIDDEN, 512], MMDT, kind="ExternalInput").ap()
    bq8 = nc.dram_tensor("bq8", [128, MT], F32, kind="ExternalInput").ap()
    bk4 = nc.dram_tensor("bk4", [128, MT], F32, kind="ExternalInput").ap()
    bv4 = nc.dram_tensor("bv4", [128, MT], F32, kind="ExternalInput").ap()
    outT = nc.dram_tensor("outT", [512, SEQ], F32, kind="ExternalOutput").ap()

    with tile.TileContext(nc) as tc, ExitStack() as ctx:
        consts = ctx.enter_context(tc.tile_pool(name="consts", bufs=1))
        bq8_sb = consts.tile([128, MT], F32)
        nc.sync.dma_start(bq8_sb[:], bq8[:])
        bk4_sb = consts.tile([128, MT], F32)
        nc.sync.dma_start(bk4_sb[:], bk4[:])
        bv4_sb = consts.tile([128, MT], F32)
        nc.sync.dma_start(bv4_sb[:], bv4[:])

        qk = ctx.enter_context(tc.tile_pool(name="qk", bufs=1))
        qT_sb = qk.tile([128, MT, SEQ], MMDT, tag="q")
        kT_sb = qk.tile([128, MT, SEQ], MMDT, tag="k")
        v_sb = qk.tile([128, ST, HPC, HEAD + 1], MMDT, tag="v")
        ones_sb = consts.tile([128, ST * HPC], F32)
        nc.gpsimd.memset(ones_sb[:], 1.0)
        nc.vector.tensor_copy(
            v_sb[:, :, :, HEAD:HEAD + 1],
            ones_sb[:].rearrange("p (a b c) -> p a b c", a=ST, b=HPC, c=1))

        # input tiles: one tile per k-chunk so matmuls only depend on the
        # DMA that actually feeds them (first matmul starts ~2us in)
        xt_pool = ctx.enter_context(tc.tile_pool(name="xt", bufs=1))
        w_pool = ctx.enter_context(tc.tile_pool(name="w", bufs=1))
        import os as _os
        _split = not _os.environ.get("K_MONO_TILES")
        wts = {}

        def load_w(which, dram):
            # weight DMAs trigger from the GpSimd queue so they land in
            # parallel with the sync-queue xT triggers below
            tiles = []
            for k in range(KT):
                t = w_pool.tile([128, 512], MMDT, name=f"w{which}{k}",
                                tag=f"w{which}{k}")
                for c in range(2):
                    nc.gpsimd.dma_start(
                        t[:, c * 256:(c + 1) * 256],
                        dram[k * 128:(k + 1) * 128,
                             c * 256:(c + 1) * 256])
                tiles.append(t)
            wts[which] = tiles

        load_w("q", wqT)
        load_w("k", wkT)
        load_w("v", wvT)

        xt = []
        for k in range(KT):
            t = xt_pool.tile([128, SEQ], MMDT, name=f"xt{k}", tag=f"xt{k}")
            for c in range(4):
                nc.sync.dma_start(
                    t[:, c * 512:(c + 1) * 512],
                    xT[k * 128:(k + 1) * 128, c * 512:(c + 1) * 512])
            xt.append(t)

        def proj_copy(which, m, n, ps):
            dst = qT_sb if which == "q" else kT_sb
            if which == "q":
                nc.vector.tensor_scalar(
                    dst[:, m, n * 512:(n + 1) * 512], ps[:],
                    0.125, bq8_sb[:, m:m + 1], ALU.mult, ALU.add)
            else:
                nc.vector.tensor_scalar_add(
                    dst[:, m, n * 512:(n + 1) * 512], ps[:],
                    bk4_sb[:, m:m + 1])

        # lead-in: q/k projections for pair 0, then all of v (pp8 scope,
        # released before the attention psum pools allocate)
        with tc.tile_pool(name="pp8", bufs=8, space="PSUM") as pp8:
            for which in ("q", "k"):
                w_t = wts[which]
                pss = []
                for n in range(NI):
                    pss.append(pp8.tile([128, 512], F32, tag="pp8",
                                        name=f"pp{which}0n{n}"))
                for k in range(KT):
                    for n in range(NI):
                        nc.tensor.matmul(
                            pss[n][:], w_t[k][:, 0:128],
                            xt[k][:, n * 512:(n + 1) * 512],
                            start=(k == 0), stop=(k == KT - 1))
                for n in range(NI):
                    proj_copy(which, 0, n, pss[n])
            for wave in (list(range(8)), list(range(8, ST))):
                pss = {}
                for s in wave:
                    pss[s] = pp8.tile([128, 512], F32, tag="pp8",
                                      name=f"ppv{s}")
                for k in range(KT):
                    for s in wave:
                        nc.tensor.matmul(
                            pss[s][:], xt[k][:, s * 128:(s + 1) * 128],
                            wts["v"][k][:],
                            start=(k == 0), stop=(k == KT - 1))
                for s in wave:
                    nc.vector.tensor_copy(
                        v_sb[:, s, :, 0:HEAD],
                        pss[s][:].rearrange("p (h d) -> p h d", h=HPC))

        # remaining projections, to be interleaved into the attention stream:
        # (which, m) blocks in the order they are needed
        import os
        pending = []
        for m in range(1, MT):
            pending.append(("q", m))
            pending.append(("k", m))
        _no_interleave = bool(os.environ.get("K_NO_INTERLEAVE"))

        es_pool = ctx.enter_context(tc.tile_pool(name="es", bufs=4))
        tail_pool = ctx.enter_context(tc.tile_pool(name="tail", bufs=2))
        sab_ps = ctx.enter_context(
            tc.tile_pool(name="sab", bufs=2, space="PSUM"))
        av_ps = ctx.enter_context(
            tc.tile_pool(name="avp", bufs=1, space="PSUM"))
        pp2 = ctx.enter_context(tc.tile_pool(name="pp2", bufs=2, space="PSUM"))


        def proj_block(which, m):
            # one m-tile of q/k projection as 2 k-outer waves of 2 psum
            # groups (weights shared between the 2 matmuls of each k step)
            w_t = wts[which]
            for nw in (0, 2):
                pss = []
                for n in (nw, nw + 1):
                    pss.append(pp2.tile([128, 512], F32, tag="pp2",
                                        name=f"p2{which}{m}n{n}"))
                for k in range(KT):
                    for d, n in enumerate((nw, nw + 1)):
                        nc.tensor.matmul(
                            pss[d][:], w_t[k][:, m * 128:(m + 1) * 128],
                            xt[k][:, n * 512:(n + 1) * 512],
                            start=(k == 0), stop=(k == KT - 1))
                for d, n in enumerate((nw, nw + 1)):
                    proj_copy(which, m, n, pss[d])

        if _no_interleave:
            while pending:
                _w, _m = pending.pop(0)
                proj_block(_w, _m)

        for p in range(MT):          # head pair (2p, 2p+1)
            for i in range(NI):      # q chunk of 512
                av = av_ps.tile([128, 1024], F32, tag="av")
                for j in range(NJ):  # k chunk of 128
                    sab = sab_ps.tile([128, 1024], F32, tag="sab")
                    nc.tensor.matmul(
                        sab[:, 0:512],
                        kT_sb[0:64, p, j * 128:(j + 1) * 128],
                        qT_sb[0:64, p, i * 512:(i + 1) * 512],
                        start=True, stop=True)
                    nc.tensor.matmul(
                        sab[:, 512:1024],
                        kT_sb[64:128, p, j * 128:(j + 1) * 128],
                        qT_sb[64:128, p, i * 512:(i + 1) * 512],
                        start=True, stop=True)
                    es = es_pool.tile([128, 1024], MMDT, tag="es")
                    nc.scalar.activation(es[:], sab[:], AF.Exp)
                    st, sp = (j == 0), (j == NJ - 1)
                    nc.tensor.matmul(
                        av[0:65, 0:512], v_sb[:, j, 2 * p, :],
                        es[:, 0:512], start=st, stop=sp)
                    nc.tensor.matmul(
                        av[0:65, 512:1024], v_sb[:, j, 2 * p + 1, :],
                        es[:, 512:1024], start=st, stop=sp)
                # tail: evacuate av to SBUF immediately so the psum bank
                # frees for the next i-chunk; everything else is SBUF-side
                av_sb = tail_pool.tile([65, 1024], F32, tag="avsb")
                nc.vector.tensor_copy(av_sb[:], av[0:65, :])
                # custom DVE / gpsimd ops require partition-0 inputs on HW:
                # stage the sums row into its own tile first
                sums = tail_pool.tile([1, 1024], F32, tag="sums")
                nc.vector.tensor_copy(sums[:], av_sb[64:65, :])
                recip = tail_pool.tile([1, 1024], F32, tag="recip")
                nc.vector.reciprocal_approx_fast(recip[:], sums[:])
                bcast = tail_pool.tile([64, 1024], F32, tag="bcast")
                nc.gpsimd.partition_broadcast(bcast[:], recip[:], channels=64)
                outm = tail_pool.tile([64, 1024], F32, tag="outm")
                nc.vector.tensor_tensor(outm[:], av_sb[0:64, :], bcast[:],
                                        ALU.mult)
                outf = tail_pool.tile([64, 1024], F32, tag="outf")
                nc.vector.tensor_scalar_add(outf[:, 0:512], outm[:, 0:512],
                                            bv4_sb[0:64, p:p + 1])
                nc.vector.tensor_scalar_add(outf[:, 512:1024],
                                            outm[:, 512:1024],
                                            bv4_sb[64:128, p:p + 1])
                nc.sync.dma_start(
                    outT[p * 128:p * 128 + 64, i * 512:(i + 1) * 512],
                    outf[:, 0:512])
                nc.sync.dma_start(
                    outT[p * 128 + 64:(p + 1) * 128, i * 512:(i + 1) * 512],
                    outf[:, 512:1024])
                # inject one pending projection block between i-chunks
                if pending and i % 2 == 1:
                    which, m = pending.pop(0)
                    proj_block(which, m)

    nc.compile()
    return nc


def kernel(x, Wq, bq, Wk, bk, Wv, bv):
    import ml_dtypes
    from concourse.bass_utils import run_bass_kernel_spmd

    mmdt = np.float32 if _DTYPE_FALLBACK else ml_dtypes.bfloat16

    x = np.asarray(x, dtype=np.float32)
    Wq = np.asarray(Wq, dtype=np.float32)
    bq = np.asarray(bq, dtype=np.float32)
    Wk = np.asarray(Wk, dtype=np.float32)
    bk = np.asarray(bk, dtype=np.float32)
    Wv = np.asarray(Wv, dtype=np.float32)
    bv = np.asarray(bv, dtype=np.float32)

    if "nc" not in _CACHE:
        _CACHE["nc"] = _build()
    nc = _CACHE["nc"]

    xTs = [np.ascontiguousarray(x[b].T).astype(mmdt) for b in range(B)]
    wT = {}
    for g in range(2):
        sl = slice(g * 512, (g + 1) * 512)
        wT[g] = (
            np.ascontiguousarray(Wq[sl, :].T).astype(mmdt),
            np.ascontiguousarray(Wk[sl, :].T).astype(mmdt),
            np.ascontiguousarray(Wv[sl, :].T).astype(mmdt),
            np.ascontiguousarray((bq[sl] * 0.125).reshape(MT, 128).T),
            np.ascontiguousarray(bk[sl].reshape(MT, 128).T),
            np.ascontiguousarray(bv[sl].reshape(MT, 128).T),
        )

    in_maps = []
    for c in range(NCORES):
        b, g = c // 2, c % 2
        wq_t, wk_t, wv_t, bq8, bk4, bv4 = wT[g]
        in_maps.append({
            "xT": xTs[b], "wqT": wq_t, "wkT": wk_t, "wvT": wv_t,
            "bq8": bq8, "bk4": bk4, "bv4": bv4,
        })

    res = run_bass_kernel_spmd(nc, in_maps, list(range(NCORES)),
                               **_CACHE.get("run_kwargs", {}))
    _CACHE["last_result"] = res

    out = np.empty((B, SEQ, HIDDEN), dtype=np.float32)
    for c in range(NCORES):
        b, g = c // 2, c % 2
        out[b, :, g * 512:(g + 1) * 512] = res.results[c]["outT"].T
    return out
